# revision 3
# baseline (speedup 1.0000x reference)
"""MixAttention Trainium2 kernel (8-core SPMD, Bass/Tile).

Sharding: (batch, query-chunk) over 8 cores: core = 4*b + qp handles batch b,
queries [qp*576, (qp+1)*576) of N=2304 (n = h*48 + w order). Each core computes
both cross-attentions (all 8 heads) for its query slice; K/V for the full
sequence are computed redundantly per core (tiny). The final 1x1-conv fuse +
gelu is fully local per core; the host only concatenates slices.

Head layout: head h = 4*g + j lives on partition strip 32*j (+0..7, row 8 =
softmax denominator), group g selects the free-dim half. S matmuls rotate over
PE row-quadrants via partition-rotated copies of kt/qt (KSH[s][32i+d,:] =
kt[32((i+s)%4)+d,:], s=0 is kt itself): slot ks9 uses shift s=ks9%3 at quadrant
(j-s)%4, giving 3-way row-tiled concurrency with only 2 extra strip copies per
tensor (cheap rectangular DMAs) instead of per-head replicas.

Device math (validated vs reference, bf16 rel err ~4e-3 < 2e-2 gate):
  - host casts inputs/weights to bf16; all projections run bf16 (1 cyc/row)
  - softmax scale folded into the q-projection weights host-side
  - depth conv1x1+bias+relu as a rank-2 matmul (ones row carries the bias)
  - bilinear 2x upsample (align_corners=False) via shifted weighted adds (DVE,
    bf16, scalar_tensor_tensor fusion)
  - S^T chunks into PSUM supertiles, exp on ScalarE, no max subtraction
    (|scaled scores| < ~8 for these inputs)
  - ones column per head in V* makes the attention matmul emit the softmax
    denominator; denominators broadcast back over channels with a tiny
    constant matmul (e4 built by memsets)
  - output projection + compress conv fused into strip-layout matmuls with
    host-precomputed weights, bias applied inside the exact-Gelu activation
"""

import numpy as np
import ml_dtypes

import bass_rust
import concourse.bass as bass
import concourse.mybir as mybir
import concourse.tile as tile
from concourse.bass_utils import run_bass_kernel_spmd
from concourse.vector_clock import ScopedClock

F32 = mybir.dt.float32
BF16 = mybir.dt.bfloat16
MM_DT = mybir.dt.bfloat16
AF = mybir.ActivationFunctionType
ALU = mybir.AluOpType

C = 64          # channels
H = 8           # heads
HD = 8          # head dim
N = 2304        # sequence (48*48)
NQ = 576        # queries per core
QC = 144        # query chunk
NQC = NQ // QC  # 4 chunks per core
KT = 128        # key tile
NKT = N // KT   # 18 key tiles
SCALE = float(HD) ** -0.5

# wpack [65, PW] bf16 free-dim section offsets
WK = {"r": 0, "d": 512}          # wk at WK[m], wq at WK[m]+256 (each [64, 256])
WQ = {"r": 256, "d": 768}
WVS = {"r": 1024, "d": 1096}     # [65, 72]
WEXPB = 1168                     # [2, 64]
XDEP = 1232                      # [2, 576] row1 = ones
PW = 1808
# wfpack [128, 385] f32 sections
WF = {"r": 0, "d": 128}          # [128, 128] each
BIASP = 256                      # [64, 1]
E4 = 257                         # [4, 128] denominator-broadcast matrix
WFW = 385


class _TileContext(tile.TileContext):
    """TileContext whose kernel-tail drain splits its semaphore waits across
    separate SP instructions (this walrus build rejects >1 wait per inst)."""

    def _drain_and_barrier(self, tick_clock, wait_clock):
        nc = self.nc
        drain_inst = nc.sync.drain()
        wait_clock.add_sem_waits(
            drain_inst.ins, ScopedClock({None: tick_clock.global_clock})
        )
        nc.all_engine_barrier()
        popped = nc._tile_sem_poison_stack.pop()
        assert popped is self._sem_poison
        nc.clear_and_free_semaphores(list(self.sems.allocated().values()))
        nc.all_engine_barrier()
        _split_multi_waits(nc)


def _split_multi_waits(nc):
    """This walrus build allows one sync wait per instruction (two for
    EventSemaphore). Hoist extra waits onto same-engine nops placed just
    before the over-subscribed instruction."""
    for f in nc.m.functions:
        for bb in f.blocks:
            insts = bb.instructions
            out = []
            changed = False
            for ins in list(insts):
                si = getattr(ins, "sync_info", None)
                waits = list(si.on_wait) if si is not None else []
                cap = 2 if isinstance(ins, mybir.InstEventSemaphore) else 1
                if len(waits) <= cap:
                    out.append(ins)
                    continue
                changed = True
                for w in waits[:-cap]:
                    nop = nc.engines[ins.engine].nop()
                    cb = nc.cur_bb.bb.instructions
                    assert cb[-1] is nop.ins
                    cb.pop()
                    nop.ins.sync_info = bass_rust.SyncInfo(on_wait=[w], on_update=[])
                    out.append(nop.ins)
                ins.sync_info = bass_rust.SyncInfo(
                    on_wait=waits[-cap:], on_update=list(si.on_update)
                )
                out.append(ins)
            if changed:
                insts.clear()
                insts.extend(out)


def _sa_off(ks9):
    """PSUM free-dim f32 offset of local k-super-tile ks9 (0..8) inside a
    [128, 1536] 3-bank supertile. bank = ks9 % 3 so consecutive-ks triples
    land in distinct banks (3-way row-tiled concurrency)."""
    return (ks9 % 3) * 512 + (ks9 // 3) * QC


def _exp_in_ap(s_tile):
    """AP enumerating the 9 slots of a supertile in ks order (slot-major,
    bank-minor), element-order compatible with a contiguous [128, 9*QC]
    output."""
    v = s_tile[:].rearrange("p (b s) -> p b s", s=512)[:, :, 0 : 3 * QC]
    return v.rearrange("p b (sl q) -> p b sl q", q=QC).transpose([0, 2, 1, 3])


MERGED_DENS = True   # single 3-dim-AP DMA for the 8 denominator rows


def build_nc(repeat=1):
    nc = bass.Bass()

    xrgb_d = nc.declare_dram_parameter("xrgb", [C + 1, N], BF16, isOutput=False)
    wpack_d = nc.declare_dram_parameter("wpack", [C + 1, PW], BF16, isOutput=False)
    wfpack_d = nc.declare_dram_parameter("wfpack", [KT, WFW], F32, isOutput=False)
    qoff_d = nc.declare_dram_parameter("qoff", [1, 1], mybir.dt.uint32, isOutput=False)
    y_d = nc.declare_dram_parameter("y", [C, NQ], F32, isOutput=True)

    with _TileContext(nc) as tc:
        with tc.tile_pool(name="const", bufs=1) as cpool:
            # ---- loads: SP carries xrgb+wpack, Act carries wfpack+qoff ----
            wpack = cpool.tile([C + 1, PW], BF16)
            nc.sync.dma_start(wpack[:], wpack_d[:])
            xrgb = cpool.tile([C + 1, N], BF16)
            nc.sync.dma_start(xrgb[:], xrgb_d[:])
            wfpack = cpool.tile([KT, WFW], F32)
            nc.scalar.dma_start(wfpack[:], wfpack_d[:])
            qoff = cpool.tile([1, 1], mybir.dt.uint32)
            nc.scalar.dma_start(qoff[:], qoff_d[:])

            dep = cpool.tile([C + 1, N], BF16)  # row 64 = ones
            nc.gpsimd.memset(dep[C : C + 1, :], 1.0)

            # ---- dynamic per-core query offset ----
            regs = nc.alloc_registers()
            nc.regs_load(regs, qoff[0:1, 0:1])
            q0v = nc.snap(regs, donate=True, min_val=0, max_val=N - NQ)
            xq = cpool.tile([C, NQ], BF16)
            nc.vector.tensor_copy(xq[:], xrgb[0:C, bass.ds(q0v, NQ)])

            kt = {}
            qt = {}
            vstar = {}
            ksh = {}
            qsh = {}
            dep_q = cpool.tile([C, NQ], BF16)

            with (
                tc.tile_pool(name="ppsum", bufs=2, space="PSUM") as ppool,
                tc.tile_pool(name="work", bufs=1) as wpool,
            ):
                # ---- depth pipeline: conv1x1+bias -> relu -> 2x bilinear ----
                dps = ppool.tile([C, 576], F32, tag="p576", bufs=1)
                nc.tensor.matmul(
                    dps[:, 0:512],
                    wpack[0:2, WEXPB : WEXPB + C],
                    wpack[0:2, XDEP : XDEP + 512],
                    start=True,
                    stop=True,
                )
                nc.tensor.matmul(
                    dps[:, 512:576],
                    wpack[0:2, WEXPB : WEXPB + C],
                    wpack[0:2, XDEP + 512 : XDEP + 576],
                    start=True,
                    stop=True,
                )
                Rm = wpool.tile([C, 576], BF16)
                nc.scalar.activation(Rm[:], dps[:], AF.Relu)

                # upsample w': [64, 24, 24] -> A [64, 24, 48]
                A = wpool.tile([C, 24 * 48], BF16)
                t75 = wpool.tile([C, 576], BF16)
                nc.vector.tensor_scalar_mul(t75[:], Rm[:], 0.75)
                R3 = Rm[:].rearrange("p (h w) -> p h w", w=24)
                T3 = t75[:].rearrange("p (h w) -> p h w", w=24)
                Av = A[:].rearrange("p (h j t) -> p h j t", j=24, t=2)
                nc.vector.scalar_tensor_tensor(
                    Av[:, :, 1:, 0], R3[:, :, 0:23], 0.25, T3[:, :, 1:],
                    ALU.mult, ALU.add,
                )
                nc.vector.tensor_copy(Av[:, :, 0:1, 0], R3[:, :, 0:1])
                nc.vector.scalar_tensor_tensor(
                    Av[:, :, 0:23, 1], R3[:, :, 1:], 0.25, T3[:, :, 0:23],
                    ALU.mult, ALU.add,
                )
                nc.vector.tensor_copy(Av[:, :, 23:24, 1], R3[:, :, 23:24])

                # upsample h': A [64, 24, 48] -> dep[0:64] as [64, 48, 48]
                u75 = wpool.tile([C, 24 * 48], BF16)
                nc.vector.tensor_scalar_mul(u75[:], A[:], 0.75)
                A3 = A[:].rearrange("p (h w) -> p h w", w=48)
                U3 = u75[:].rearrange("p (h w) -> p h w", w=48)
                Bv = dep[0:C, :].rearrange("p (i t w) -> p i t w", t=2, w=48)
                nc.vector.scalar_tensor_tensor(
                    Bv[:, 1:, 0, :], A3[:, 0:23, :], 0.25, U3[:, 1:, :],
                    ALU.mult, ALU.add,
                )
                nc.vector.tensor_copy(Bv[:, 0:1, 0, :], A3[:, 0:1, :])
                nc.vector.scalar_tensor_tensor(
                    Bv[:, 0:23, 1, :], A3[:, 1:, :], 0.25, U3[:, 0:23, :],
                    ALU.mult, ALU.add,
                )
                nc.vector.tensor_copy(Bv[:, 23:24, 1, :], A3[:, 23:24, :])

                nc.vector.tensor_copy(dep_q[:], dep[0:C, bass.ds(q0v, NQ)])

                # ---- projections into strip layout (all bf16) ----
                ncopy = 0
                for m, xin, qin in (("r", xrgb, xq), ("d", dep, dep_q)):
                    kt[m] = cpool.tile([KT, 2 * N], MM_DT, tag=f"kt_{m}", name=f"kt_{m}")
                    for g in range(2):
                        for c0 in range(0, N, 512):
                            cw = min(512, N - c0)
                            kp = ppool.tile([KT, 512], F32, tag="kp", name="kp")
                            nc.tensor.matmul(
                                kp[:, 0:cw],
                                wpack[0:C, WK[m] + g * KT : WK[m] + (g + 1) * KT],
                                xin[0:C, c0 : c0 + cw],
                                start=True,
                                stop=True,
                            )
                            dst = kt[m][:, g * N + c0 : g * N + c0 + cw]
                            if ncopy % 2 == 0:
                                nc.scalar.copy(dst, kp[:, 0:cw])
                            else:
                                nc.vector.tensor_copy(dst, kp[:, 0:cw])
                            ncopy += 1
                    qt[m] = cpool.tile([KT, 2 * NQ], MM_DT, tag=f"qt_{m}", name=f"qt_{m}")
                    for g in range(2):
                        for c0 in range(0, NQ, 288):
                            qp_ = ppool.tile([KT, 512], F32, tag="kp", name="qp_")
                            nc.tensor.matmul(
                                qp_[:, 0:288],
                                wpack[0:C, WQ[m] + g * KT : WQ[m] + (g + 1) * KT],
                                qin[:, c0 : c0 + 288],
                                start=True,
                                stop=True,
                            )
                            dst = qt[m][:, g * NQ + c0 : g * NQ + c0 + 288]
                            if ncopy % 2 == 0:
                                nc.scalar.copy(dst, qp_[:, 0:288])
                            else:
                                nc.vector.tensor_copy(dst, qp_[:, 0:288])
                            ncopy += 1
                    vstar[m] = cpool.tile(
                        [KT, NKT * 72], MM_DT, tag=f"vs_{m}", name=f"vs_{m}"
                    )
                    for si, nsl in ((0, 7), (1, 7), (2, 4)):
                        vps = ppool.tile([KT, 504], F32, tag="kp", name="vps")
                        for u in range(nsl):
                            ks = 7 * si + u
                            nc.tensor.matmul(
                                vps[:, u * 72 : (u + 1) * 72],
                                xin[:, ks * KT : (ks + 1) * KT],
                                wpack[0 : C + 1, WVS[m] : WVS[m] + 72],
                                start=True,
                                stop=True,
                            )
                        dst = vstar[m][:, si * 504 : si * 504 + nsl * 72]
                        if ncopy % 2 == 0:
                            nc.scalar.copy(dst, vps[:, 0 : nsl * 72])
                        else:
                            nc.vector.tensor_copy(dst, vps[:, 0 : nsl * 72])
                        ncopy += 1

                    # ---- partition-rotated strip copies for S rotation ----
                    ksh[m] = {0: kt[m]}
                    qsh[m] = {0: qt[m]}
                    ndma = 0
                    for s in (1, 2):
                        ksh[m][s] = cpool.tile(
                            [KT, 2 * N], MM_DT, tag=f"ksh{s}_{m}", name=f"ksh{s}_{m}"
                        )
                        qsh[m][s] = cpool.tile(
                            [KT, 2 * NQ], MM_DT, tag=f"qsh{s}_{m}", name=f"qsh{s}_{m}"
                        )
                        for i in range(4):
                            j = (i + s) % 4
                            eng = nc.sync if ndma % 2 == 0 else nc.scalar
                            eng.dma_start(
                                ksh[m][s][32 * i : 32 * i + 8, :],
                                kt[m][32 * j : 32 * j + 8, :],
                            )
                            eng2 = nc.scalar if ndma % 2 == 0 else nc.sync
                            eng2.dma_start(
                                qsh[m][s][32 * i : 32 * i + 8, :],
                                qt[m][32 * j : 32 * j + 8, :],
                            )
                            ndma += 1

            # ---- attention ----
            import contextlib
            rep_ctx = tc.For_i(0, repeat, 1) if repeat > 1 else contextlib.nullcontext()
            with (
                tc.tile_pool(name="sa", bufs=1, space="PSUM") as sapool,
                tc.tile_pool(name="sb", bufs=1, space="PSUM") as sbpool,
                tc.tile_pool(name="av", bufs=2, space="PSUM") as avpool,
                tc.tile_pool(name="att", bufs=2) as apool,
                rep_ctx,
            ):
                for qc in range(NQC):
                    qsl = slice(qc * QC, (qc + 1) * QC)
                    xt = {}
                    for m, kv in (("r", "d"), ("d", "r")):
                        vm = vstar[kv]
                        xnum = apool.tile([KT, 2 * QC], F32, tag="xnum", name="xnum")
                        nc.vector.memset(xnum[:], 0.0)
                        av = avpool.tile([KT, 2 * QC], F32, tag="avt", name="av")
                        for hg in range(2):
                            heads = [4 * hg + j for j in range(4)]
                            pts = {}
                            for h in heads:
                                g, j = divmod(h, 4)
                                pt = apool.tile(
                                    [KT, NKT * QC], MM_DT, tag="pt", name="pt", bufs=5
                                )
                                pts[h] = pt
                                for half, spool in ((0, sapool), (1, sbpool)):
                                    st = spool.tile(
                                        [KT, 1536], F32, tag=f"s{half}", name=f"s{half}"
                                    )
                                    for ks9 in range(9):
                                        ks = half * 9 + ks9
                                        o = _sa_off(ks9)
                                        s = ks9 % 3
                                        strip = 32 * ((j - s) % 4)
                                        nc.tensor.matmul(
                                            st[:, o : o + QC],
                                            ksh[kv][s][strip : strip + 8, g * N + ks * KT : g * N + (ks + 1) * KT],
                                            qsh[m][s][strip : strip + 8, g * NQ + qc * QC : g * NQ + (qc + 1) * QC],
                                            start=True,
                                            stop=True,
                                            tile_position=(strip, 0),
                                        )
                                    nc.scalar.activation(
                                        pt[:, half * 9 * QC : (half + 1) * 9 * QC],
                                        _exp_in_ap(st),
                                        AF.Exp,
                                    )
                            for ks in range(NKT):
                                for h in heads:
                                    j = h % 4
                                    nc.tensor.matmul(
                                        av[32 * j : 32 * j + 9, hg * QC : (hg + 1) * QC],
                                        vm[:, ks * 72 + 9 * h : ks * 72 + 9 * h + 9],
                                        pts[h][:, ks * QC : (ks + 1) * QC],
                                        start=(ks == 0),
                                        stop=(ks == NKT - 1),
                                        tile_position=(0, 32 * j),
                                        skip_group_check=True,
                                    )
                            for h in heads:
                                j = h % 4
                                nc.vector.tensor_copy(
                                    xnum[32 * j : 32 * j + 9, hg * QC : (hg + 1) * QC],
                                    av[32 * j : 32 * j + 9, hg * QC : (hg + 1) * QC],
                                )
                        # denominators -> [4, 2*QC] (j on partitions, g in free)
                        dens4 = apool.tile([4, 2 * QC], F32, tag="dens4", name="dens4")
                        if MERGED_DENS:
                            src = xnum[:].rearrange(
                                "(j r) (g q) -> j r g q", j=4, q=QC
                            )[:, 8]
                            dst = dens4[0:4, :].rearrange("j (g q) -> j g q", q=QC)
                            nc.sync.dma_start(dst, src)
                        else:
                            for h in range(H):
                                g, j = divmod(h, 4)
                                nc.sync.dma_start(
                                    dens4[j : j + 1, g * QC : (g + 1) * QC],
                                    xnum[32 * j + 8 : 32 * j + 9, g * QC : (g + 1) * QC],
                                )
                        recd = apool.tile([4, 2 * QC], F32, tag="recd", name="recd")
                        nc.vector.reciprocal(recd[:], dens4[:])
                        denx = avpool.tile([KT, 2 * QC], F32, tag="avt", name="denx")
                        nc.tensor.matmul(
                            denx[:],
                            wfpack[0:4, E4 : E4 + KT],
                            recd[:],
                            start=True,
                            stop=True,
                        )
                        xt[m] = apool.tile(
                            [KT, 2 * QC], F32, tag=f"xt_{m}", name=f"xt_{m}"
                        )
                        nc.vector.tensor_mul(xt[m][:], xnum[:], denx[:])
                    fp = avpool.tile([C, QC], F32, tag="avt", name="fp")
                    first = True
                    for m in ("r", "d"):
                        for g in range(2):
                            nc.tensor.matmul(
                                fp[:],
                                wfpack[0:KT, WF[m] + g * C : WF[m] + (g + 1) * C],
                                xt[m][:, g * QC : (g + 1) * QC],
                                start=first,
                                stop=(m == "d" and g == 1),
                            )
                            first = False
                    outt = apool.tile([C, QC], F32, tag="outt", name="outt")
                    nc.scalar.activation(
                        outt[:], fp[:], AF.Gelu, bias=wfpack[0:C, BIASP : BIASP + 1]
                    )
                    nc.sync.dma_start(y_d[:, qsl], outt[:])

    return nc


# ---------------- host side ----------------

_BUILT = {}


def _get_nc():
    if "nc" not in _BUILT:
        _BUILT["nc"] = build_nc()
    return _BUILT["nc"]


def _bf(x):
    return np.asarray(x, np.float32).astype(ml_dtypes.bfloat16)


def _host_prep(inputs):
    """Build per-core input maps from full inputs."""
    f = lambda k: np.ascontiguousarray(np.asarray(inputs[k], np.float32))
    rgb_fea = f("rgb_fea")
    depth_fea = f("depth_fea")
    w_exp = f("w_exp")
    b_exp = f("b_exp")

    def vstar_w(w_v):
        W = np.zeros((C + 1, 72), np.float32)
        for h in range(H):
            W[0:C, 9 * h : 9 * h + 8] = w_v.T[:, 8 * h : 8 * h + 8]
            W[C, 9 * h + 8] = 1.0
        return W

    def fuse_w(Wp):
        # Wp [64 out, 64 in]; strip layout rows 32j+d = in-channel 8*(4g+j)+d
        W = np.zeros((KT, 2 * C), np.float32)
        for g in range(2):
            for j in range(4):
                h = 4 * g + j
                W[32 * j : 32 * j + 8, g * C : (g + 1) * C] = Wp[:, 8 * h : 8 * h + 8].T
        return W

    def strip_w(w):
        # lhsT [64 in, 2*128]: col g*128 + 32j+d = row 8*(4g+j)+d of w
        W = np.zeros((C, 2 * KT), np.float32)
        for g in range(2):
            for j in range(4):
                h = 4 * g + j
                W[:, g * KT + 32 * j : g * KT + 32 * j + 8] = w[8 * h : 8 * h + 8, :].T
        return W

    w_comp = f("w_comp")
    W_r, W_d = w_comp[:, :C], w_comp[:, C:]

    # wfpack: wf_r | wf_d | biasp | e4
    wfpack = np.zeros((KT, WFW), np.float32)
    wfpack[:, 0:128] = fuse_w(W_r @ f("w_rgb_proj"))
    wfpack[:, 128:256] = fuse_w(W_d @ f("w_dep_proj"))
    wfpack[0:C, 256] = W_r @ f("b_rgb_proj") + W_d @ f("b_dep_proj") + f("b_comp")
    for j in range(4):
        wfpack[j, E4 + 32 * j : E4 + 32 * j + 8] = 1.0
    wfpack = np.ascontiguousarray(wfpack)

    # wpack template (batch-independent sections)
    wpack = np.zeros((C + 1, PW), np.float32)
    wpack[0:C, WK["r"] : WK["r"] + 256] = strip_w(f("w_rgb_k"))
    wpack[0:C, WQ["r"] : WQ["r"] + 256] = strip_w(f("w_rgb_q") * SCALE)
    wpack[0:C, WK["d"] : WK["d"] + 256] = strip_w(f("w_dep_k"))
    wpack[0:C, WQ["d"] : WQ["d"] + 256] = strip_w(f("w_dep_q") * SCALE)
    wpack[0 : C + 1, WVS["r"] : WVS["r"] + 72] = vstar_w(f("w_rgb_v"))
    wpack[0 : C + 1, WVS["d"] : WVS["d"] + 72] = vstar_w(f("w_dep_v"))
    wpack[0, WEXPB : WEXPB + C] = w_exp.ravel()
    wpack[1, WEXPB : WEXPB + C] = b_exp.ravel()
    wpack[1, XDEP : XDEP + 576] = 1.0

    ones = np.ones((1, N), np.float32)
    in_maps = []
    wpack_b = {}
    for b in range(2):
        wp = wpack.copy()
        wp[0, XDEP : XDEP + 576] = depth_fea[b, 0].reshape(576)
        wpack_b[b] = np.ascontiguousarray(_bf(wp))
    for core in range(8):
        b, qp = divmod(core, 4)
        m = {
            "xrgb": np.ascontiguousarray(
                _bf(np.vstack([rgb_fea[b].reshape(C, N), ones]))
            ),
            "wpack": wpack_b[b],
            "wfpack": wfpack,
            "qoff": np.array([[qp * NQ]], dtype=np.uint32),
        }
        in_maps.append(m)
    return in_maps


def _assemble(results):
    out = np.zeros((2, C, 48, 48), np.float32)
    for core in range(8):
        b, qp = divmod(core, 4)
        y = results[core]["y"]
        out[b, :, qp * 12 : (qp + 1) * 12, :] = y.reshape(C, 12, 48)
    # (c, h, w) -> reference order (c, w, h)
    return np.ascontiguousarray(out.transpose(0, 1, 3, 2))


def kernel(**inputs):
    nc = _get_nc()
    in_maps = _host_prep(inputs)
    res = run_bass_kernel_spmd(nc, in_maps, list(range(8)))
    return _assemble(res.results)


def run_sim_core(inputs, core=0):
    """CoreSim single-core debug path (not used by the harness)."""
    from concourse import bass_interp

    nc = build_nc()
    sim = bass_interp.CoreSim(nc)
    in_map = _host_prep(inputs)[core]
    for k, v in in_map.items():
        sim.tensor(k)[:] = v
    sim.simulate()
    return np.array(sim.tensor("y"))
